# revision 46
# baseline (speedup 1.0000x reference)
"""EqualizedModConv2D (StyleGAN2 modulated conv) on 8 TRN2 NeuronCores.

Winograd F(2x2, 3x3) formulation (exact algebra; error only from f16
operand rounding, measured ~6e-4 L2 vs the f32 reference):
    mod[n,i]  = style[n] @ (fc_weight * fc_scale).T[.,i] + bias[i] + 1
    out[n]    = demod_eff[n,:] * winograd_conv(mod[n,:] * x[n], weight)
    demod_eff[n,o] = 1 / sqrt( sum_i mod[n,i]^2 * wsq[o,i] + eps/w_scale^2 )

Winograd: per 4x4 input tile D (stride 2), U = Bt D B (16 points, all
+-1 combos), M_p = W_p^T U_p (PE matmuls, W_p = (G w G^T)_p precomputed
on host), Y = At M A (2x2 outputs per tile, +-1 combos).  This cuts PE
matmul work 2.25x vs direct conv (256 vs 576 N=512 matmuls per core).

Sharding: data-parallel over batch N=16 -> 2 samples per core; weights
replicated.

Engine assignment:
  PE  : mod/demod matmuls + 256 Winograd point matmuls (f16, N=512)
  DVE : input-transform pass1 (rows, 2x packed mode), pass2 for column
        c=0 (startup critical path), PSUM drains (inverse rows pass),
        inverse pass2 (quadrant combine), reciprocal for demod
  GPS : border memsets, input-transform pass2 for columns 1..3
  ACT : modulation multiply, demod sqrt, output scale (+quadrant
        interleave) and output DMAs on its own ring
Loop order is c-outer (all 4 oc-blocks per Winograd column) so each
transform column is produced once and consumed 4x back-to-back.
"""

import numpy as np

import concourse.bass as bass
import concourse.bacc as bacc
import concourse.tile as tile
from concourse import mybir
from concourse.bass_utils import run_bass_kernel_spmd

F32 = mybir.dt.float32
F16 = mybir.dt.float16
AF = mybir.ActivationFunctionType

N_FULL, IC, OC, H, W = 16, 512, 512, 32, 32
DLAT, KS = 512, 3
NCORES = 8
NPC = N_FULL // NCORES          # samples per core
HP, WP = H + 2, W + 4           # padded image rows 34, cols 36 (2 pad cols
                                # keep every row-slice 4B aligned for DVE 2x)
FC_SCALE = 1.0 / float(np.sqrt(DLAT))
EPS_EFF = 1e-8 * (IC * KS * KS)  # eps / w_scale^2
NIB = IC // 128
NOB = OC // 128
NDB = DLAT // 128
NT = 16                         # winograd tiles per image dim (32/2)

_NC = None


def _dedup_ldweights(nc):
    """Drop InstLdweights that reload the stationary weights already in the
    PE array (same weights AP as the previous load, nothing clobbering the
    array in between, no sync attached)."""
    from concourse import mybir as _mb
    removed = 0
    for blk in nc.m.functions[0].blocks:
        insts = blk.instructions
        keep = []
        last_ld_key = None
        for i in insts:
            tn = type(i).__name__
            if tn == "InstLdweights":
                key = str(i.ins[0])
                si = i.sync_info
                clean = si is None or (len(si.on_wait) == 0 and
                                       len(si.on_update) == 0)
                if key == last_ld_key and clean:
                    removed += 1
                    continue
                last_ld_key = key
            elif tn in ("InstMatmult", "InstEventSemaphore"):
                pass
            elif getattr(i, "engine", None) == _mb.EngineType.PE:
                last_ld_key = None
            keep.append(i)
        if len(keep) != len(insts):
            insts[:] = keep
    return removed


def _build(loop_iters=None, unroll=1):
    nc = bacc.Bacc()
    # host-packed layouts (see _make_in_maps): partition dim always 128
    x_d = nc.declare_dram_parameter("x", [NIB, 128, NPC * H * W], F16, False)
    # wt: winograd-transformed weights, [ocb][128 ic, (c, icb, r, 128 oc)]
    wt_d = nc.declare_dram_parameter("wt", [NOB, 128, 4 * NIB * 4 * 128],
                                     F16, False)
    # pk packs per d-block: [fcwT (512c) | styleT (NPC c) | bias (1c)]
    PKC = IC + NPC + 1
    pk_d = nc.declare_dram_parameter("pk", [128, NDB * PKC], F16, False)
    wsq_d = nc.declare_dram_parameter("wsq", [128, NIB * NOB * 128], F16, False)
    out_d = nc.declare_dram_parameter("out", [NPC, OC, H, W], F32, True)

    import contextlib
    with tile.TileContext(nc) as tc:
        with (tc.For_i(0, loop_iters, 1,
                       hint_engines=(mybir.EngineType.PE,
                                     mybir.EngineType.Activation,
                                     mybir.EngineType.DVE,
                                     mybir.EngineType.Pool,
                                     mybir.EngineType.SP))
              if loop_iters else contextlib.nullcontext()):
         with (
            tc.tile_pool(name="const", bufs=1) as cpool,
            tc.tile_pool(name="xi", bufs=4) as xi_pool,
            tc.tile_pool(name="xpad", bufs=2) as xpad_pool,
            tc.tile_pool(name="rp", bufs=4) as r_pool,
            tc.tile_pool(name="up", bufs=16) as u_pool,
            tc.tile_pool(name="wtp", bufs=8) as wt_pool,
            tc.tile_pool(name="yp", bufs=1) as y_pool,
            tc.tile_pool(name="outsb", bufs=2) as out_pool,
            tc.tile_pool(name="small", bufs=4) as small_pool,
            tc.tile_pool(name="tmp", bufs=1) as tmp_pool,
            tc.tile_pool(name="cpsum", bufs=8, space="PSUM") as cpsum_pool,
        ):
          for _it in range(unroll):
            # ------- input DMAs on the SP ring, consumer order --------------
            pkt = cpool.tile([128, NDB * PKC], F16, tag="pk", name="pk")
            nc.sync.dma_start(out=pkt[:], in_=pk_d[:, :])
            fcw_sb = [pkt[:, d * PKC:d * PKC + IC] for d in range(NDB)]
            st_sb = [pkt[:, d * PKC + IC:d * PKC + IC + NPC]
                     for d in range(NDB)]

            # x DMAs right behind pk: they gate the whole transform chain
            xis = []
            for i in range(NIB):
                xi = xi_pool.tile([128, NPC * H * W], F16, tag="xi",
                                  name=f"xi{i}")
                nc.sync.dma_start(out=xi[:], in_=x_d[i, :, :])
                xis.append(xi)

            wsq_sb = cpool.tile([128, NIB * NOB * 128], F16, tag="wsq",
                                name="wsq")
            nc.sync.dma_start(out=wsq_sb[:], in_=wsq_d[:, :])

            b1_sb = []
            for d in range(NDB):
                t1 = cpool.tile([128, 1], F32, tag=f"b1{d}", name=f"b1{d}")
                nc.vector.tensor_scalar_add(
                    t1[:], pkt[:, d * PKC + IC + NPC:d * PKC + IC + NPC + 1],
                    1.0)
                b1_sb.append(t1)
            eps_sb = cpool.tile([128, 1], F32, tag="eps", name="eps")
            nc.vector.memset(eps_sb[:], float(EPS_EFF))

            # ---------------- mod / mod^2  (i on partitions, n free) --------
            sp_m = cpsum_pool.tile([128, 512], F32, tag="cps", name="sp_m")
            sp_d = cpsum_pool.tile([128, 512], F32, tag="cps", name="sp_d")
            sp_w = cpsum_pool.tile([128, 512], F32, tag="cps", name="sp_w")
            mod_sb, mod2_sb = [], []
            for i in range(NIB):
                mp = sp_m[:, i * NPC:(i + 1) * NPC]
                for d in range(NDB):
                    nc.tensor.matmul(
                        mp,
                        fcw_sb[d][:, i * 128:(i + 1) * 128],
                        st_sb[d],
                        start=(d == 0),
                        stop=(d == NDB - 1),
                    )
                m = cpool.tile([128, NPC], F32, tag=f"mod{i}", name=f"mod{i}")
                nc.scalar.activation(m[:], mp, AF.Identity,
                                     bias=b1_sb[i][:, 0:1], scale=FC_SCALE)
                m2 = cpool.tile([128, NPC], F16, tag=f"mod2{i}",
                                name=f"mod2{i}")
                nc.scalar.square(m2[:], m[:])
                mod_sb.append(m)
                mod2_sb.append(m2)

            # ---------------- demod for ALL oc blocks upfront ---------------
            dems = []
            for o in range(NOB):
                dp = sp_d[:, o * NPC:(o + 1) * NPC]
                for i in range(NIB):
                    nc.tensor.matmul(
                        dp,
                        wsq_sb[:, (i * NOB + o) * 128:(i * NOB + o + 1) * 128],
                        mod2_sb[i][:], start=(i == 0), stop=(i == NIB - 1))
                sq = small_pool.tile([128, NPC], F32, tag="sq", name=f"sq{o}")
                nc.scalar.activation(sq[:], dp, AF.Sqrt,
                                     bias=eps_sb[:, 0:1], scale=1.0)
                dem = small_pool.tile([128, NPC], F32, tag="dem",
                                      name=f"dem{o}")
                nc.vector.reciprocal(dem[:], sq[:])
                dems.append(dem)

            # PE warmup: dummy matmuls into a never-read psum bank keep the
            # PE HAM clock gate at 8/8 (2.4 GHz) while the input transform
            # runs; without them the PE idles >3.4us and the first conv
            # column executes throttled at 1.2 GHz.  Same stationary weights
            # throughout -> all but one ldweights dedup away.
            for w in range(60):
                nc.tensor.matmul(sp_w[:, 0:128], fcw_sb[0][:, 0:128],
                                 pkt[:, 0:128], start=True, stop=True)

            # ------- weight DMAs, c-major so column c weights arrive early --
            wtc = {}

            def dma_wt(o, c):
                t = wt_pool.tile([128, NIB * 4 * 128], F16, tag="wt",
                                 name=f"wt_o{o}c{c}")
                nc.sync.dma_start(out=t[:], in_=wt_d[o, :, c * 2048:
                                                     (c + 1) * 2048])
                wtc[(o, c)] = t

            for o in range(NOB):
                dma_wt(o, 0)

            # ------- x: load, zero-pad + modulate, input transform ----------
            # xp: [128, n, 34 rows, 36 cols]; R: [128, n*4+r, 16, 36]
            # u[c][icb]: [128, n*4+r, 16 ty, 16 tx]
            us = [[None] * NIB for _ in range(4)]
            Rs = []
            for i in range(NIB):
                xi = xis[i]
                # xp cols are parity-de-interleaved: flat col p*18+j holds
                # image col 2j+p.  Keeps every pass2 operand contiguous
                # (DVE 2x packed mode); the de-interleave itself rides the
                # flat-rate ACT modulation multiply.
                xp = xpad_pool.tile([128, NPC, HP, WP], F16, tag="xp",
                                    name=f"xp{i}")
                nc.gpsimd.memset(xp[:, :, 0, :], 0.0)          # row 0
                nc.gpsimd.memset(xp[:, :, HP - 1, :], 0.0)     # row 33
                nc.gpsimd.memset(xp[:, :, 1:HP - 1, 0:1], 0.0)   # col 0
                nc.gpsimd.memset(xp[:, :, 1:HP - 1, 34:35], 0.0)  # col 33
                nc.gpsimd.memset(xp[:, :, :, 17:18], 0.0)      # pad col 34
                nc.gpsimd.memset(xp[:, :, :, 35:36], 0.0)      # pad col 35
                for n in range(NPC):
                    xr = xi[:, n * H * W:(n + 1) * H * W].rearrange(
                        "p (a b) -> p a b", b=W)
                    # modulate: ACT takes n=0, gpsimd n=1 (parallel lanes;
                    # both tolerate the parity de-interleave strides)
                    if n == 0:
                        nc.scalar.mul(xp[:, n, 1:H + 1, 1:17],
                                      xr[:, :, 1::2], mod_sb[i][:, n:n + 1])
                        nc.scalar.mul(xp[:, n, 1:H + 1, 18:34],
                                      xr[:, :, 0::2], mod_sb[i][:, n:n + 1])
                    else:
                        nc.gpsimd.tensor_scalar_mul(
                            xp[:, n, 1:H + 1, 1:17], xr[:, :, 1::2],
                            mod_sb[i][:, n:n + 1])
                        nc.gpsimd.tensor_scalar_mul(
                            xp[:, n, 1:H + 1, 18:34], xr[:, :, 0::2],
                            mod_sb[i][:, n:n + 1])
                # pass1 (rows): R_r = Bt-row combos, DVE 2x packed mode
                R = r_pool.tile([128, NPC * 4, NT, WP], F16, tag="R",
                                name=f"R{i}")
                e0 = xp[:, :, 0:HP - 3:2, :]   # rows 0,2,..,30
                e2 = xp[:, :, 2:HP - 1:2, :]   # rows 2,4,..,32
                o1 = xp[:, :, 1:HP - 2:2, :]   # rows 1,3,..,31
                o3 = xp[:, :, 3:HP:2, :]       # rows 3,5,..,33
                nc.vector.tensor_sub(R[:, 0::4, :, :], e0, e2)
                nc.vector.tensor_add(R[:, 1::4, :, :], o1, e2)
                nc.vector.tensor_sub(R[:, 2::4, :, :], e2, o1)
                nc.vector.tensor_sub(R[:, 3::4, :, :], o1, o3)
                Rs.append(R)
            # pass2 (cols): all contiguous slices thanks to the parity
            # layout -> DVE 2x packed mode for every op.  Emitted c-major
            # so column c is complete as early as possible (conv consumes
            # columns in order); c2/c3 ride the otherwise-idle gpsimd.
            for c, (eng, fn, a0, b0) in enumerate([
                    (nc.vector, "tensor_sub", 0, 1),    # ce0 - ce2
                    (nc.vector, "tensor_add", 18, 1),   # co1 + ce2
                    (nc.gpsimd, "tensor_sub", 1, 18),   # ce2 - co1
                    (nc.gpsimd, "tensor_sub", 18, 19)]):  # co1 - co3
                for i in range(NIB):
                    R = Rs[i]
                    u = u_pool.tile([128, NPC * 4, NT, NT], F16, tag="u",
                                    name=f"u{i}c{c}")
                    getattr(eng, fn)(u[:, :, :, :],
                                     R[:, :, :, a0:a0 + 16],
                                     R[:, :, :, b0:b0 + 16])
                    us[c][i] = u

            # ---------------- winograd point matmuls, c-outer ---------------
            # y[o] quadrant slots (i*2+s): the inverse column pass At =
            # [[1,1,1,0],[0,1,-1,-1]] is accumulated in place per column:
            #   s=0 slot: +P_c0 +P_c1 +P_c2 ; s=1 slot: +P_c1 -P_c2 -P_c3
            ys = [y_pool.tile([128, 4, 512], F16, tag=f"y{o}", name=f"y{o}")
                  for o in range(NOB)]
            for c in range(4):
                # last column runs o3 first so its output chain (drain ->
                # inverse -> scale -> DMA) overlaps the remaining groups
                oorder = [3, 0, 1, 2] if c == 3 else list(range(NOB))
                if c + 1 < 4:
                    for o in ([3, 0, 1, 2] if c + 1 == 3 else range(NOB)):
                        dma_wt(o, c + 1)
                for o in oorder:
                    pc = [cpsum_pool.tile([128, 512], F32, tag="cps",
                                          name=f"pc_c{c}o{o}r{r}")
                          for r in range(4)]
                    for r in range(4):
                        for i in range(NIB):
                            lw = wtc[(o, c)][:, (i * 4 + r) * 128:
                                             (i * 4 + r + 1) * 128]
                            nc.tensor.matmul(pc[r][:], lw,
                                             us[c][i][:, r::4, :, :],
                                             start=(i == 0),
                                             stop=(i == NIB - 1))
                    # drain: inverse rows pass  P0=M0+M1+M2, P1=M1-M2-M3.
                    # ACT copies the twice-read M1/M2 banks to SBUF f16 so
                    # half the DVE chain runs in 2x packed mode.
                    S = tmp_pool.tile([128, 512], F16, tag="S", name="S")
                    D = tmp_pool.tile([128, 512], F16, tag="D", name="D")
                    y = ys[o]
                    if c < 3:
                        # ACT stages M1/M2 in SBUF f16 so the S/D combines
                        # run in DVE 2x packed mode
                        M1s = tmp_pool.tile([128, 512], F16, tag="M1s",
                                            name="M1s")
                        M2s = tmp_pool.tile([128, 512], F16, tag="M2s",
                                            name="M2s")
                        nc.scalar.copy(M1s[:], pc[1][:])
                        nc.scalar.copy(M2s[:], pc[2][:])
                        nc.vector.tensor_add(S[:], M1s[:], M2s[:])
                        nc.vector.tensor_sub(D[:], M1s[:], M2s[:])
                    else:
                        # final column: single ACT hop (one PSUM operand per
                        # DVE op is a hardware requirement, NCC_IBVF027)
                        M1s = tmp_pool.tile([128, 512], F16, tag="M1s",
                                            name="M1s")
                        nc.scalar.copy(M1s[:], pc[1][:])
                        nc.vector.tensor_add(S[:], M1s[:], pc[2][:])
                        nc.vector.tensor_sub(D[:], M1s[:], pc[2][:])
                    # P_i for this column land in (or combine into) y slots
                    if c == 0:
                        nc.vector.tensor_add(y[:, 0, :], pc[0][:], S[:])
                        nc.vector.tensor_sub(y[:, 2, :], D[:], pc[3][:])
                    elif c == 1:
                        nc.vector.tensor_add(y[:, 1, :], pc[0][:], S[:])
                        nc.vector.tensor_add(y[:, 0, :], y[:, 0, :],
                                             y[:, 1, :])
                        nc.vector.tensor_sub(y[:, 3, :], D[:], pc[3][:])
                        nc.vector.tensor_add(y[:, 2, :], y[:, 2, :],
                                             y[:, 3, :])
                    else:
                        T0 = tmp_pool.tile([128, 512], F16, tag="T0",
                                           name="T0")
                        T1 = tmp_pool.tile([128, 512], F16, tag="T1",
                                           name="T1")
                        nc.vector.tensor_add(T0[:], pc[0][:], S[:])
                        nc.vector.tensor_sub(T1[:], D[:], pc[3][:])
                        if c == 2:  # s=0 gets +P_c2; s=1 gets -P_c2
                            nc.vector.tensor_add(y[:, 0, :], y[:, 0, :],
                                                 T0[:])
                            nc.vector.tensor_add(y[:, 2, :], y[:, 2, :],
                                                 T1[:])
                        nc.vector.tensor_sub(y[:, 1, :], y[:, 1, :], T0[:])
                        nc.vector.tensor_sub(y[:, 3, :], y[:, 3, :], T1[:])

                    if c == 3:
                        # ---------------- scale + store for ocb o -----------
                        ob = out_pool.tile([128, NPC, H * W], F32, tag="ob",
                                           name=f"ob{o}")
                        for n in range(NPC):
                            obv = ob[:, n, :].rearrange("p (a b) -> p a b",
                                                        b=W)
                            for i in range(2):
                                # both s-quadrants in one flat-rate ACT op:
                                # in iterated (ty, tx, s) matches out rows
                                # i::2 with s interleaved in x
                                yv = y[:, i * 2:i * 2 + 2,
                                       n * 256:(n + 1) * 256].rearrange(
                                           "p s (a b) -> p a b s", b=NT)
                                ov = obv[:, i::2, :].rearrange(
                                    "p a (b s) -> p a b s", s=2)
                                nc.scalar.mul(ov, yv, dems[o][:, n:n + 1])
                            # SP ring is idle once inputs are loaded; using
                            # it for outputs keeps the ACT queue free for
                            # the drain copies + scales
                            nc.sync.dma_start(
                                out=out_d[n, o * 128:(o + 1) * 128,
                                          :, :].rearrange("p a b -> p (a b)"),
                                in_=ob[:, n, :],
                            )
    nc.finalize()
    _dedup_ldweights(nc)
    return nc


def _get_nc():
    global _NC
    if _NC is None:
        _NC = _build()
    return _NC


def _make_in_maps(x, style, weight, fc_weight, bias):
    x16 = np.asarray(x, np.float32).astype(np.float16)
    w64 = np.asarray(weight, np.float64)
    # winograd weight transform W_p = G w G^T  (host, f64 -> f16)
    G = np.array([[1, 0, 0], [0.5, 0.5, 0.5], [0.5, -0.5, 0.5], [0, 0, 1]],
                 np.float64)
    Wt = np.einsum("ab,oibc,dc->adoi", G, w64, G)  # (4a, 4d, OC, IC)
    # -> [ocb, 128 ic, (d=c, icb, a=r, 128 oc)]
    wt = np.ascontiguousarray(
        Wt.reshape(4, 4, NOB, 128, NIB, 128)
        .transpose(2, 5, 1, 4, 0, 3)
        .reshape(NOB, 128, 4 * NIB * 4 * 128)).astype(np.float16)
    # wsqT[ic, oc] -> [128ic_p, (i, o, c)]
    wsqT = (w64 ** 2).sum(axis=(2, 3)).T.astype(np.float16)
    wsq = np.ascontiguousarray(
        wsqT.reshape(NIB, 128, NOB, 128).transpose(1, 0, 2, 3)
        .reshape(128, NIB * NOB * 128))
    styleT = np.asarray(style, np.float32).T
    fcwT = np.asarray(fc_weight, np.float32).T
    biasr = np.asarray(bias, np.float32).reshape(IC, 1)
    in_maps = []
    for c in range(NCORES):
        pk0 = np.concatenate(
            [fcwT, styleT[:, c * NPC:(c + 1) * NPC], biasr],
            axis=1).astype(np.float16)
        pk = np.ascontiguousarray(
            pk0.reshape(NDB, 128, IC + NPC + 1).transpose(1, 0, 2)
            .reshape(128, NDB * (IC + NPC + 1)))
        xc = np.ascontiguousarray(
            x16[c * NPC:(c + 1) * NPC].reshape(NPC, NIB, 128, H * W)
            .transpose(1, 2, 0, 3).reshape(NIB, 128, NPC * H * W))
        in_maps.append({
            "x": xc,
            "wt": wt,
            "pk": pk,
            "wsq": wsq,
        })
    return in_maps


def _run(in_maps, trace=False):
    last = None
    for _ in range(3):
        try:
            return run_bass_kernel_spmd(_get_nc(), in_maps, list(range(NCORES)),
                                        trace=trace)
        except Exception as e:  # transient NRT/device errors: retry
            last = e
    raise last


def kernel(x, style, weight, fc_weight, bias):
    br = _run(_make_in_maps(x, style, weight, fc_weight, bias))
    out = np.concatenate([br.results[c]["out"] for c in range(NCORES)], axis=0)
    return out


def _make_runner(nc, in_maps):
    import jax
    import numpy as np
    from jax.sharding import Mesh, PartitionSpec
    from jax.experimental.shard_map import shard_map
    from concourse import mybir as _mb
    from concourse.bass2jax import (_bass_exec_p, install_neuronx_cc_hook,
                                    partition_id_tensor)
    install_neuronx_cc_hook()
    n_cores = len(in_maps)
    partition_name = nc.partition_id_tensor.name if nc.partition_id_tensor else None
    in_names, out_names, out_avals, zero_outs = [], [], [], []
    for alloc in nc.m.functions[0].allocations:
        if not isinstance(alloc, _mb.MemoryLocationSet):
            continue
        name = alloc.memorylocations[0].name
        if alloc.kind == "ExternalInput":
            if name != partition_name:
                in_names.append(name)
        elif alloc.kind == "ExternalOutput":
            shape = tuple(alloc.tensor_shape)
            dtype = _mb.dt.np(alloc.dtype)
            out_avals.append(jax.core.ShapedArray(shape, dtype))
            out_names.append(name)
            zero_outs.append(np.zeros(shape, dtype))
    n_params = len(in_names)
    all_in_names = list(in_names) + list(out_names)
    if partition_name is not None:
        all_in_names.append(partition_name)

    def _body(*args):
        operands = list(args)
        if partition_name is not None:
            operands.append(partition_id_tensor())
        outs = _bass_exec_p.bind(
            *operands,
            out_avals=tuple(out_avals),
            in_names=tuple(all_in_names),
            out_names=tuple(out_names),
            lowering_input_output_aliases=(),
            sim_require_finite=True,
            sim_require_nnan=True,
            nc=nc,
        )
        return tuple(outs)

    devices = jax.devices()[:n_cores]
    mesh = Mesh(np.asarray(devices), ("core",))
    in_specs = (PartitionSpec("core"),) * (n_params + len(out_names))
    out_specs = (PartitionSpec("core"),) * len(out_names)
    fn = jax.jit(shard_map(_body, mesh=mesh, in_specs=in_specs,
                           out_specs=out_specs, check_rep=False))
    concat = []
    for nm in in_names:
        per = [np.asarray(in_maps[c][nm]) for c in range(n_cores)]
        concat.append(np.concatenate(per, axis=0))
    concat += [np.zeros((n_cores * z.shape[0], *z.shape[1:]), z.dtype)
               for z in zero_outs]
    args = [jax.device_put(a) for a in concat]
    return fn, args


def _time_runner(fn, args, iters, reps):
    import time
    import jax
    o = fn(*args)
    jax.block_until_ready(o)  # compile + warm
    best = float("inf")
    for _ in range(reps):
        t0 = time.perf_counter()
        for _ in range(iters):
            o = fn(*args)
            jax.block_until_ready(o)
        best = min(best, (time.perf_counter() - t0) / iters)
    return best


_NC_LOOPS = {}
_LOOP_R1 = 16
_LOOP_R = 272


def measure_hw(inputs, iters=6, reps=2, trials=12):
    """Differential HW timing between two hardware-loop builds:
    (wall(body x R2) - wall(body x R1)) / (R2 - R1)."""
    in_maps = _make_in_maps(**inputs)
    for r in (_LOOP_R1, _LOOP_R):
        if r not in _NC_LOOPS:
            _NC_LOOPS[r] = _build(loop_iters=r)
    fn1, args1 = _make_runner(_NC_LOOPS[_LOOP_R1], in_maps)
    fnR, argsR = _make_runner(_NC_LOOPS[_LOOP_R], in_maps)
    import time as _time
    best = (float("inf"), (0.0, 0.0))
    for t in range(trials):
        if t:
            _time.sleep(8)
        w1 = _time_runner(fn1, args1, iters, reps) * 1e9
        wR = _time_runner(fnR, argsR, iters, reps) * 1e9
        per = (wR - w1) / (_LOOP_R - _LOOP_R1)
        if per < best[0]:
            best = (per, (w1, wR))
    return best


def predict_ns():
    """Cost-model (TimelineSim) predicted single-core kernel duration in ns."""
    from concourse.timeline_sim import TimelineSim
    ts = TimelineSim(_get_nc(), no_exec=True)
    return ts.simulate()


def run_profiled(inputs):
    """Dev helper: run with NTFF tracing; returns BassKernelResults."""
    return _run(_make_in_maps(**inputs), trace=True)
